# revision 18
# baseline (speedup 1.0000x reference)
"""Trainium2 Bass kernel for pointer-network greedy decode (sparse_attention).

Problem: B=256 batches, N=384 nodes, D=128, H*Hd=128. Sequential greedy
decode with visited masking, tanh-clipped bilinear scores.

Key algebraic reformulation: scores sum over all heads/dims, so
    raw[b,n] = ctx[b]^T (qv_flat kv_flat^T) hvec[b,n]
with ctx = hbar + h_last + h_first.  Precompute per-batch pairwise matrix
    A[b] = (hvec[b] @ qf_sc) @ (hvec[b] @ kf)^T     (qf_sc = 0.25*qf)
Then each decode step is:  x = base[b] + A[b][a_last,:] + A[b][a_first,:]
(an indirect-DMA row gather + adds instead of matmuls).

Selection must reproduce XLA-CPU fp32 tanh tie semantics: tanh(x)==1.0 iff
x >= L_SAT (=7.9988117f). Selection rule (validated bit-exact vs reference
on the problem seed): first unvisited index with x >= T, where
    T = min(max_unvisited(x), L_SAT),  or -1e6 if max <= -L_SAT
Masking is additive (-1e9 into the persistent bf tile).

Performance structure:
- batches split into G_GROUPS interleaved groups; one group's DVE compute
  overlaps the other group's indirect-DMA gather latency.
- tanh/exp for logp are batched K_BATCH steps at a time (ACT table loads
  otherwise dominate); ln/tanh-of-max batched once at the end.
- mask/index bookkeeping on gpsimd, selection chain on DVE, logp on ACT.

Sharding: pure data-parallel over batch, 8 cores x 32 batches.
"""

import numpy as np

import concourse.bass as bass
import concourse.bacc as bacc
import concourse.mybir as mybir
from concourse.bass import IndirectOffsetOnAxis
from concourse.bass_utils import run_bass_kernel_spmd
from concourse.masks import make_identity
from concourse.tile import TileContext

F32 = mybir.dt.float32
U32 = mybir.dt.uint32
I32 = mybir.dt.int32
U8 = mybir.dt.uint8

B_FULL = 256
N_CORES = 8
BL = B_FULL // N_CORES  # 32 batches per core
N = 384
D = 128

G_GROUPS = 2
GB = BL // G_GROUPS     # batches per group
K_BATCH = 8             # steps per ACT tanh batch

L_SAT = 7.9988117  # np.float32: smallest x with XLA-cpu tanh(x) == 1.0
NEG_BIG = -1.0e9
NEG_MED = -1.0e6

AX_X = mybir.AxisListType.X
Alu = mybir.AluOpType
Act = mybir.ActivationFunctionType


def build_nc(n_steps: int = N, compile: bool = True) -> bass.Bass:
    assert n_steps % K_BATCH == 0
    nc = bacc.Bacc()

    hvec_in = nc.dram_tensor("hvec", [BL, N, D], F32, kind="ExternalInput")
    qf_in = nc.dram_tensor("qf", [D, D], F32, kind="ExternalInput")   # pre-scaled by 0.25
    kf_in = nc.dram_tensor("kf", [D, D], F32, kind="ExternalInput")
    qh2_in = nc.dram_tensor("qh2", [D, 2 * BL], F32, kind="ExternalInput")

    pi_out = nc.dram_tensor("pi", [BL, N], I32, kind="ExternalOutput")
    logp_out = nc.dram_tensor("logp", [BL, 1], F32, kind="ExternalOutput")

    a_dram = nc.dram_tensor("a_mat", [BL * N, N], F32)
    bases_dram = nc.dram_tensor("bases", [BL, 2, N], F32)

    with TileContext(nc) as tc:
        with (
            tc.tile_pool(name="const", bufs=1) as cpool,
            tc.tile_pool(name="p1", bufs=2) as p1,
            tc.tile_pool(name="psum", bufs=2, space="PSUM") as psum,
            tc.tile_pool(name="loop", bufs=3) as lp,
            tc.tile_pool(name="xmb", bufs=2) as xmb,
            tc.tile_pool(name="pers", bufs=1) as pers,
        ):
            # ---------------- constants ----------------
            ident = cpool.tile([128, 128], F32, tag="ident")
            make_identity(nc, ident[:])
            qf_sb = cpool.tile([D, D], F32, tag="qf")
            nc.sync.dma_start(out=qf_sb[:], in_=qf_in[:])
            kf_sb = cpool.tile([D, D], F32, tag="kf")
            nc.sync.dma_start(out=kf_sb[:], in_=kf_in[:])
            qh2_sb = cpool.tile([D, 2 * BL], F32, tag="qh2")
            nc.sync.dma_start(out=qh2_sb[:], in_=qh2_in[:])

            # ---------------- phase 1: A = Qsc @ K^T per batch ----------------
            for b in range(BL):
                hv = p1.tile([128, 3, 128], F32, tag="hv")
                nc.sync.dma_start(out=hv[:], in_=hvec_in[b].rearrange("(c p) d -> p c d", p=128))
                hvT = p1.tile([128, N], F32, tag="hvT")
                for c in range(3):
                    tps = psum.tile([128, 128], F32, tag="tps")
                    nc.tensor.transpose(out=tps[:], in_=hv[:, c, :], identity=ident[:])
                    nc.vector.tensor_copy(out=hvT[:, c * 128:(c + 1) * 128], in_=tps[:])
                qt_ps = psum.tile([128, N], F32, tag="qkps")
                nc.tensor.matmul(out=qt_ps[:], lhsT=qf_sb[:], rhs=hvT[:], start=True, stop=True)
                qt = p1.tile([128, N], F32, tag="qt")
                nc.scalar.copy(out=qt[:], in_=qt_ps[:])
                kt_ps = psum.tile([128, N], F32, tag="qkps")
                nc.tensor.matmul(out=kt_ps[:], lhsT=kf_sb[:], rhs=hvT[:], start=True, stop=True)
                kt = p1.tile([128, N], F32, tag="kt")
                nc.scalar.copy(out=kt[:], in_=kt_ps[:])
                for c in range(3):
                    a_ps = psum.tile([128, N], F32, tag="aps")
                    nc.tensor.matmul(out=a_ps[:], lhsT=qt[:, c * 128:(c + 1) * 128],
                                     rhs=kt[:], start=True, stop=True)
                    a_sb = p1.tile([128, N], F32, tag="asb")
                    nc.vector.tensor_copy(out=a_sb[:], in_=a_ps[:])
                    nc.sync.dma_start(out=a_dram[b * N + c * 128: b * N + (c + 1) * 128, :],
                                      in_=a_sb[:])
                b_ps = psum.tile([2, N], F32, tag="bps")
                nc.tensor.matmul(out=b_ps[:], lhsT=qh2_sb[:, 2 * b:2 * b + 2],
                                 rhs=kt[:], start=True, stop=True)
                b_sb = p1.tile([2, N], F32, tag="bsb")
                nc.vector.tensor_copy(out=b_sb[:], in_=b_ps[:])
                nc.sync.dma_start(out=bases_dram[b], in_=b_sb[:])

            # ---------------- shared decode constants ----------------
            iota_u32 = pers.tile([GB, N], U32, tag="iotau")
            nc.gpsimd.iota(iota_u32[:], pattern=[[1, N]], base=0, channel_multiplier=0)
            iota_row = pers.tile([GB, N], F32, tag="iota")
            nc.vector.tensor_copy(out=iota_row[:], in_=iota_u32[:])
            ones8 = pers.tile([GB, 8], F32, tag="ones8")
            nc.vector.memset(ones8[:], 1.0)
            neg1e6 = pers.tile([GB, 1], F32, tag="neg1e6")
            nc.vector.memset(neg1e6[:], NEG_MED)
            negone = pers.tile([GB, 1], F32, tag="negone")
            nc.vector.memset(negone[:], -1.0)
            e10 = pers.tile([GB, 1], F32, tag="e10")          # device's exp(-10)
            nc.scalar.activation(out=e10[:], in_=negone[:], func=Act.Exp, scale=10.0)
            # per-step visited-count correction row: corr[:, t] = t * e10
            corr = pers.tile([GB, N], F32, tag="corr")
            nc.vector.tensor_scalar(out=corr[:], in0=iota_row[:], scalar1=e10[:, 0:1],
                                    scalar2=None, op0=Alu.mult)

            class Grp:
                pass

            grps = []
            for g in range(G_GROUPS):
                G = Grp()
                G.g = g
                G.rowbase = pers.tile([GB, 1], U32, tag=f"rowb{g}")
                nc.gpsimd.iota(G.rowbase[:], pattern=[[0, 1]], base=g * GB * N,
                               channel_multiplier=N)
                G.base = pers.tile([GB, N], F32, tag=f"base{g}")
                nc.sync.dma_start(out=G.base[:], in_=bases_dram[g * GB:(g + 1) * GB, 0, :])
                G.bf = pers.tile([GB, N], F32, tag=f"bf{g}")
                G.pi = pers.tile([GB, N], I32, tag=f"pi{g}")
                nc.vector.memset(G.pi[:], 0)
                G.m1parts = pers.tile([GB, N], F32, tag=f"m1p{g}")
                G.sscparts = pers.tile([GB, N], F32, tag=f"ssc{g}")
                G.xmbuf = None     # current K-step xm buffer (rotated)
                G.idx = None
                G.aidx = None
                G.gt = None
                grps.append(G)

            def new_xmbuf(G):
                G.xmbuf = xmb.tile([GB, K_BATCH * N], F32, tag=f"xmbuf{G.g}")

            def xm_ap(G, t):
                o = (t % K_BATCH) * N
                return G.xmbuf[:, o:o + N]

            def emit_select(G, t):
                """DVE selection chain for step t (xm already written)."""
                xm = xm_ap(G, t)
                m1 = G.m1parts[:, t:t + 1]
                nc.vector.tensor_reduce(out=m1, in_=xm, axis=AX_X, op=Alu.max)
                thr = lp.tile([GB, 1], F32, tag=f"thr{G.g}")
                nc.vector.tensor_scalar(out=thr[:], in0=m1, scalar1=L_SAT,
                                        scalar2=None, op0=Alu.min)
                negsat = lp.tile([GB, 1], U8, tag=f"negsat{G.g}")
                nc.vector.tensor_scalar(out=negsat[:], in0=m1, scalar1=-L_SAT,
                                        scalar2=None, op0=Alu.is_le)
                nc.vector.copy_predicated(out=thr[:], mask=negsat[:], data=neg1e6[:])
                y = lp.tile([GB, N], F32, tag=f"y{G.g}")
                nc.vector.tensor_scalar(out=y[:], in0=xm, scalar1=thr[:],
                                        scalar2=None, op0=Alu.is_ge)
                aidx = lp.tile([GB, 8], U32, tag=f"aidx{G.g}")
                nc.vector.max_index(out=aidx[:], in_max=ones8[:], in_values=y[:])
                G.aidx = aidx

            def emit_postselect(G, t, n_steps, mask_bf=True):
                """gpsimd bookkeeping after selection of step t: next-gather idx,
                pi write, mask update."""
                aidx = G.aidx
                if t < n_steps - 1:
                    idx = lp.tile([GB, 1], U32, tag=f"idx{G.g}")
                    nc.gpsimd.tensor_tensor(out=idx[:], in0=aidx[:, 0:1],
                                            in1=G.rowbase[:], op=Alu.add)
                    g_t = lp.tile([GB, N], F32, tag=f"g{G.g}")
                    nc.gpsimd.indirect_dma_start(
                        out=g_t[:], out_offset=None, in_=a_dram[:],
                        in_offset=IndirectOffsetOnAxis(ap=idx[:, :1], axis=0))
                    G.gt_next = g_t
                nc.gpsimd.tensor_copy(out=G.pi[:, t:t + 1], in_=aidx[:, 0:1])
                if mask_bf and t < n_steps - 1:
                    af = lp.tile([GB, 1], F32, tag=f"af{G.g}")
                    nc.gpsimd.tensor_copy(out=af[:], in_=aidx[:, 0:1])
                    eqf = lp.tile([GB, N], F32, tag=f"eqf{G.g}")
                    nc.gpsimd.tensor_scalar(out=eqf[:], in0=iota_row[:], scalar1=af[:, 0:1],
                                            scalar2=NEG_BIG, op0=Alu.is_equal, op1=Alu.mult)
                    nc.vector.tensor_tensor(out=G.bf[:], in0=G.bf[:], in1=eqf[:], op=Alu.add)

            def emit_act_batch(G, k, n_steps):
                """tanh+exp for steps [k*K_BATCH, (k+1)*K_BATCH) from G.xmbuf."""
                t0 = k * K_BATCH
                nsub = min(K_BATCH, n_steps - t0)
                th = xmb.tile([GB, K_BATCH * N], F32, tag=f"th{G.g}")
                nc.scalar.activation(out=th[:, 0:nsub * N], in_=G.xmbuf[:, 0:nsub * N],
                                     func=Act.Tanh)
                for j in range(nsub):
                    ex = lp.tile([GB, N], F32, tag=f"ex{G.g}")
                    nc.scalar.activation(out=ex[:], in_=th[:, j * N:(j + 1) * N],
                                         func=Act.Exp, scale=10.0,
                                         accum_out=G.sscparts[:, t0 + j:t0 + j + 1])

            # ---------------- decode ----------------
            for G in grps:
                new_xmbuf(G)
                # x0 straight into xmbuf slot 0
                nc.sync.dma_start(out=xm_ap(G, 0), in_=bases_dram[G.g * GB:(G.g + 1) * GB, 1, :])
            for G in grps:
                emit_select(G, 0)
                emit_postselect(G, 0, n_steps, mask_bf=False)
                G.aidx0 = G.aidx

            for t in range(1, n_steps):
                for G in grps:
                    if t % K_BATCH == 0:
                        emit_act_batch(G, t // K_BATCH - 1, n_steps)
                        new_xmbuf(G)
                    g_t = G.gt_next
                    xm = xm_ap(G, t)
                    if t == 1:
                        # bf = base + G(a0);  bf[a0] += -1e9 came from eqf of step0?
                        # step0's eqf was computed against bf before it existed -> do here
                        nc.vector.tensor_tensor(out=G.bf[:], in0=G.base[:], in1=g_t[:],
                                                op=Alu.add)
                        af0 = lp.tile([GB, 1], F32, tag=f"af{G.g}")
                        nc.gpsimd.tensor_copy(out=af0[:], in_=G.aidx0[:, 0:1])
                        eqf0 = lp.tile([GB, N], F32, tag=f"eqf{G.g}")
                        nc.gpsimd.tensor_scalar(out=eqf0[:], in0=iota_row[:],
                                                scalar1=af0[:, 0:1], scalar2=NEG_BIG,
                                                op0=Alu.is_equal, op1=Alu.mult)
                        nc.vector.tensor_tensor(out=G.bf[:], in0=G.bf[:], in1=eqf0[:],
                                                op=Alu.add)
                    nc.vector.tensor_tensor(out=xm, in0=G.bf[:], in1=g_t[:], op=Alu.add)
                    emit_select(G, t)
                    emit_postselect(G, t, n_steps)

            for G in grps:
                emit_act_batch(G, (n_steps - 1) // K_BATCH, n_steps)

            # ---------------- finalize logp ----------------
            for G in grps:
                ssc = lp.tile([GB, N], F32, tag=f"sscf{G.g}")
                nc.vector.tensor_tensor(out=ssc[:, 0:n_steps], in0=G.sscparts[:, 0:n_steps],
                                        in1=corr[:, 0:n_steps], op=Alu.subtract)
                lnp = lp.tile([GB, N], F32, tag=f"lnp{G.g}")
                nc.scalar.activation(out=lnp[:, 0:n_steps], in_=ssc[:, 0:n_steps], func=Act.Ln)
                thm = lp.tile([GB, N], F32, tag=f"thm{G.g}")
                nc.scalar.activation(out=thm[:, 0:n_steps], in_=G.m1parts[:, 0:n_steps],
                                     func=Act.Tanh)
                s1 = lp.tile([GB, 1], F32, tag=f"s1{G.g}")
                nc.vector.reduce_sum(out=s1[:], in_=thm[:, 0:n_steps], axis=AX_X)
                s2 = lp.tile([GB, 1], F32, tag=f"s2{G.g}")
                nc.vector.reduce_sum(out=s2[:], in_=lnp[:, 0:n_steps], axis=AX_X)
                lp_t = lp.tile([GB, 1], F32, tag=f"lpt{G.g}")
                nc.vector.tensor_scalar(out=lp_t[:], in0=s1[:], scalar1=10.0,
                                        scalar2=None, op0=Alu.mult)
                nc.vector.tensor_tensor(out=lp_t[:], in0=lp_t[:], in1=s2[:], op=Alu.subtract)
                nc.sync.dma_start(out=logp_out[G.g * GB:(G.g + 1) * GB, :], in_=lp_t[:])
                nc.sync.dma_start(out=pi_out[G.g * GB:(G.g + 1) * GB, :], in_=G.pi[:, 0:N])

    if compile:
        nc.compile()
    return nc


def _host_prep(hvec, hbar, qv_p, kv_p, vec_1, vec_f):
    """Host-side prep: scale fold + context projections + per-core input maps."""
    hvec = np.asarray(hvec, dtype=np.float32)
    hbar = np.asarray(hbar, dtype=np.float32)
    qf = np.asarray(qv_p, dtype=np.float32).reshape(D, D)
    kf = np.asarray(kv_p, dtype=np.float32).reshape(D, D)
    vec_1 = np.asarray(vec_1, dtype=np.float32)
    vec_f = np.asarray(vec_f, dtype=np.float32)

    qf_sc = (np.float32(0.25) * qf).astype(np.float32)
    qhbar_sc = (hbar @ qf_sc).astype(np.float32)
    ctx0 = ((hbar + vec_1[None, :]).astype(np.float32) + vec_f[None, :]).astype(np.float32)
    q0_sc = (ctx0 @ qf_sc).astype(np.float32)

    in_maps = []
    for c in range(N_CORES):
        sl = slice(c * BL, (c + 1) * BL)
        qh2 = np.empty((D, 2 * BL), np.float32)
        qh2[:, 0::2] = qhbar_sc[sl].T
        qh2[:, 1::2] = q0_sc[sl].T
        in_maps.append({
            "hvec": np.ascontiguousarray(hvec[sl]),
            "qf": qf_sc,
            "kf": np.ascontiguousarray(kf),
            "qh2": qh2,
        })
    return in_maps


def kernel(hvec, hbar, qv_p, kv_p, vec_1, vec_f):
    in_maps = _host_prep(hvec, hbar, qv_p, kv_p, vec_1, vec_f)
    nc = build_nc()
    res = run_bass_kernel_spmd(nc, in_maps, list(range(N_CORES)))
    pi = np.concatenate([np.asarray(r["pi"]) for r in res.results], axis=0)
    logp = np.concatenate([np.asarray(r["logp"]).reshape(-1) for r in res.results])
    return np.ascontiguousarray(pi.T.astype(np.int32)), logp.astype(np.float32)


# revision 20
# speedup vs baseline: 2.5137x; 2.5137x over previous
"""Trainium2 Bass kernel for pointer-network greedy decode (sparse_attention).

Problem: B=256 batches, N=384 nodes, D=128, H*Hd=128. Sequential greedy
decode with visited masking, tanh-clipped bilinear scores.

Key algebraic reformulation: scores sum over all heads/dims, so
    raw[b,n] = ctx[b]^T (qv_flat kv_flat^T) hvec[b,n]
with ctx = hbar + h_last + h_first.  Precompute per-batch pairwise matrix
    A[b] = (hvec[b] @ qf_sc) @ (hvec[b] @ kf)^T     (qf_sc = 0.25*qf)
Then each decode step is:  x = base[b] + A[b][a_last,:] + A[b][a_first,:]
(an indirect-DMA row gather + adds instead of matmuls).

Selection must reproduce XLA-CPU fp32 tanh tie semantics: tanh(x)==1.0 iff
x >= L_SAT (=7.9988117f). Selection rule (validated bit-exact vs reference
on the problem seed): first unvisited index with x >= T, where
    T = min(max_unvisited(x), L_SAT),  or -1e6 if max <= -L_SAT
Masking is additive (-1e9 into the persistent bf tile).

Performance structure:
- batches split into G_GROUPS interleaved groups; one group's DVE compute
  overlaps the other group's indirect-DMA gather latency.
- tanh/exp for logp are batched K_BATCH steps at a time (ACT table loads
  otherwise dominate); ln/tanh-of-max batched once at the end.
- mask/index bookkeeping on gpsimd, selection chain on DVE, logp on ACT.

Sharding: pure data-parallel over batch, 8 cores x 32 batches.
"""

import numpy as np

import concourse.bass as bass
import concourse.bacc as bacc
import concourse.mybir as mybir
from concourse.bass import IndirectOffsetOnAxis
from concourse.bass_utils import run_bass_kernel_spmd
from concourse.masks import make_identity
from concourse.tile import TileContext

F32 = mybir.dt.float32
U32 = mybir.dt.uint32
I32 = mybir.dt.int32
U8 = mybir.dt.uint8

B_FULL = 256
N_CORES = 8
BL = B_FULL // N_CORES  # 32 batches per core
N = 384
D = 128

G_GROUPS = 2
GB = BL // G_GROUPS     # batches per group
K_BATCH = 8             # steps per ACT tanh batch

L_SAT = 7.9988117  # np.float32: smallest x with XLA-cpu tanh(x) == 1.0
NEG_BIG = -1.0e9
NEG_MED = -1.0e6

AX_X = mybir.AxisListType.X
Alu = mybir.AluOpType
Act = mybir.ActivationFunctionType


def build_nc(n_steps: int = N, compile: bool = True) -> bass.Bass:
    assert n_steps % K_BATCH == 0
    nc = bacc.Bacc()

    hvec_in = nc.dram_tensor("hvec", [BL, N, D], F32, kind="ExternalInput")
    qf_in = nc.dram_tensor("qf", [D, D], F32, kind="ExternalInput")   # pre-scaled by 0.25
    kf_in = nc.dram_tensor("kf", [D, D], F32, kind="ExternalInput")
    qh2_in = nc.dram_tensor("qh2", [D, 2 * BL], F32, kind="ExternalInput")

    pi_out = nc.dram_tensor("pi", [BL, N], I32, kind="ExternalOutput")
    logp_out = nc.dram_tensor("logp", [BL, 1], F32, kind="ExternalOutput")

    a_dram = nc.dram_tensor("a_mat", [BL * N, N], F32)
    bases_dram = nc.dram_tensor("bases", [BL, 2, N], F32)

    with TileContext(nc) as tc:
        with (
            tc.tile_pool(name="const", bufs=1) as cpool,
            tc.tile_pool(name="p1", bufs=2) as p1,
            tc.tile_pool(name="psum", bufs=2, space="PSUM") as psum,
            tc.tile_pool(name="loop", bufs=3) as lp,
            tc.tile_pool(name="xmb", bufs=2) as xmb,
            tc.tile_pool(name="pers", bufs=1) as pers,
        ):
            # ---------------- constants ----------------
            ident = cpool.tile([128, 128], F32, tag="ident")
            make_identity(nc, ident[:])
            qf_sb = cpool.tile([D, D], F32, tag="qf")
            nc.sync.dma_start(out=qf_sb[:], in_=qf_in[:])
            kf_sb = cpool.tile([D, D], F32, tag="kf")
            nc.sync.dma_start(out=kf_sb[:], in_=kf_in[:])
            qh2_sb = cpool.tile([D, 2 * BL], F32, tag="qh2")
            nc.sync.dma_start(out=qh2_sb[:], in_=qh2_in[:])

            # ---------------- phase 1: A = Qsc @ K^T per batch ----------------
            for b in range(BL):
                hv = p1.tile([128, 3, 128], F32, tag="hv")
                nc.sync.dma_start(out=hv[:], in_=hvec_in[b].rearrange("(c p) d -> p c d", p=128))
                hvT = p1.tile([128, N], F32, tag="hvT")
                for c in range(3):
                    tps = psum.tile([128, 128], F32, tag="tps")
                    nc.tensor.transpose(out=tps[:], in_=hv[:, c, :], identity=ident[:])
                    nc.vector.tensor_copy(out=hvT[:, c * 128:(c + 1) * 128], in_=tps[:])
                qt_ps = psum.tile([128, N], F32, tag="qkps")
                nc.tensor.matmul(out=qt_ps[:], lhsT=qf_sb[:], rhs=hvT[:], start=True, stop=True)
                qt = p1.tile([128, N], F32, tag="qt")
                nc.scalar.copy(out=qt[:], in_=qt_ps[:])
                kt_ps = psum.tile([128, N], F32, tag="qkps")
                nc.tensor.matmul(out=kt_ps[:], lhsT=kf_sb[:], rhs=hvT[:], start=True, stop=True)
                kt = p1.tile([128, N], F32, tag="kt")
                nc.scalar.copy(out=kt[:], in_=kt_ps[:])
                for c in range(3):
                    a_ps = psum.tile([128, N], F32, tag="aps")
                    nc.tensor.matmul(out=a_ps[:], lhsT=qt[:, c * 128:(c + 1) * 128],
                                     rhs=kt[:], start=True, stop=True)
                    a_sb = p1.tile([128, N], F32, tag="asb")
                    nc.vector.tensor_copy(out=a_sb[:], in_=a_ps[:])
                    nc.sync.dma_start(out=a_dram[b * N + c * 128: b * N + (c + 1) * 128, :],
                                      in_=a_sb[:])
                b_ps = psum.tile([2, N], F32, tag="bps")
                nc.tensor.matmul(out=b_ps[:], lhsT=qh2_sb[:, 2 * b:2 * b + 2],
                                 rhs=kt[:], start=True, stop=True)
                b_sb = p1.tile([2, N], F32, tag="bsb")
                nc.vector.tensor_copy(out=b_sb[:], in_=b_ps[:])
                nc.sync.dma_start(out=bases_dram[b], in_=b_sb[:])

            # ---------------- shared decode constants ----------------
            iota_u32 = pers.tile([GB, N], U32, tag="iotau")
            nc.gpsimd.iota(iota_u32[:], pattern=[[1, N]], base=0, channel_multiplier=0)
            iota_row = pers.tile([GB, N], F32, tag="iota")
            nc.vector.tensor_copy(out=iota_row[:], in_=iota_u32[:])
            ones8 = pers.tile([GB, 8], F32, tag="ones8")
            nc.vector.memset(ones8[:], 1.0)
            neg1e6 = pers.tile([GB, 1], F32, tag="neg1e6")
            nc.vector.memset(neg1e6[:], NEG_MED)
            negone = pers.tile([GB, 1], F32, tag="negone")
            nc.vector.memset(negone[:], -1.0)
            e10 = pers.tile([GB, 1], F32, tag="e10")          # device's exp(-10)
            nc.scalar.activation(out=e10[:], in_=negone[:], func=Act.Exp, scale=10.0)
            # per-step visited-count correction row: corr[:, t] = t * e10
            corr = pers.tile([GB, N], F32, tag="corr")
            nc.vector.tensor_scalar(out=corr[:], in0=iota_row[:], scalar1=e10[:, 0:1],
                                    scalar2=None, op0=Alu.mult)

            class Grp:
                pass

            grps = []
            for g in range(G_GROUPS):
                G = Grp()
                G.g = g
                G.rowbase = pers.tile([GB, 1], U32, tag=f"rowb{g}")
                nc.gpsimd.iota(G.rowbase[:], pattern=[[0, 1]], base=g * GB * N,
                               channel_multiplier=N)
                G.base = pers.tile([GB, N], F32, tag=f"base{g}")
                nc.sync.dma_start(out=G.base[:], in_=bases_dram[g * GB:(g + 1) * GB, 0, :])
                G.bf = pers.tile([GB, N], F32, tag=f"bf{g}")
                G.pi = pers.tile([GB, N], I32, tag=f"pi{g}")
                nc.vector.memset(G.pi[:], 0)
                G.m1parts = pers.tile([GB, N], F32, tag=f"m1p{g}")
                G.sscparts = pers.tile([GB, N], F32, tag=f"ssc{g}")
                G.xmbuf = None     # current K-step xm buffer (rotated)
                G.idx = None
                G.aidx = None
                G.gt = None
                grps.append(G)

            def new_xmbuf(G):
                G.xmbuf = xmb.tile([GB, K_BATCH * N], F32, tag=f"xmbuf{G.g}")

            def xm_ap(G, t):
                o = (t % K_BATCH) * N
                return G.xmbuf[:, o:o + N]

            def emit_select(G, t):
                """DVE selection chain for step t (xm already written)."""
                xm = xm_ap(G, t)
                m1 = G.m1parts[:, t:t + 1]
                nc.vector.tensor_reduce(out=m1, in_=xm, axis=AX_X, op=Alu.max)
                thr = lp.tile([GB, 1], F32, tag=f"thr{G.g}")
                nc.vector.tensor_scalar(out=thr[:], in0=m1, scalar1=L_SAT,
                                        scalar2=None, op0=Alu.min)
                negsat = lp.tile([GB, 1], U8, tag=f"negsat{G.g}")
                nc.vector.tensor_scalar(out=negsat[:], in0=m1, scalar1=-L_SAT,
                                        scalar2=None, op0=Alu.is_le)
                nc.vector.copy_predicated(out=thr[:], mask=negsat[:], data=neg1e6[:])
                y = lp.tile([GB, N], F32, tag=f"y{G.g}")
                nc.vector.tensor_scalar(out=y[:], in0=xm, scalar1=thr[:],
                                        scalar2=None, op0=Alu.is_ge)
                aidx = lp.tile([GB, 8], U32, tag=f"aidx{G.g}")
                nc.vector.max_index(out=aidx[:], in_max=ones8[:], in_values=y[:])
                G.aidx = aidx

            def emit_postselect(G, t, n_steps, mask_bf=True):
                """gpsimd bookkeeping after selection of step t: next-gather idx,
                pi write, mask update."""
                aidx = G.aidx
                if t < n_steps - 1:
                    idx = lp.tile([GB, 1], U32, tag=f"idx{G.g}")
                    nc.gpsimd.tensor_tensor(out=idx[:], in0=aidx[:, 0:1],
                                            in1=G.rowbase[:], op=Alu.add)
                    g_t = lp.tile([GB, N], F32, tag=f"g{G.g}")
                    nc.gpsimd.indirect_dma_start(
                        out=g_t[:], out_offset=None, in_=a_dram[:],
                        in_offset=IndirectOffsetOnAxis(ap=idx[:, :1], axis=0))
                    G.gt_next = g_t
                nc.gpsimd.tensor_copy(out=G.pi[:, t:t + 1], in_=aidx[:, 0:1])
                if mask_bf and t < n_steps - 1:
                    af = lp.tile([GB, 1], F32, tag=f"af{G.g}")
                    nc.gpsimd.tensor_copy(out=af[:], in_=aidx[:, 0:1])
                    eqf = lp.tile([GB, N], F32, tag=f"eqf{G.g}")
                    nc.vector.tensor_scalar(out=eqf[:], in0=iota_row[:], scalar1=af[:, 0:1],
                                            scalar2=NEG_BIG, op0=Alu.is_equal, op1=Alu.mult)
                    nc.vector.tensor_tensor(out=G.bf[:], in0=G.bf[:], in1=eqf[:], op=Alu.add)

            def emit_act_batch(G, k, n_steps):
                """tanh+exp for steps [k*K_BATCH, (k+1)*K_BATCH) from G.xmbuf."""
                t0 = k * K_BATCH
                nsub = min(K_BATCH, n_steps - t0)
                th = xmb.tile([GB, K_BATCH * N], F32, tag=f"th{G.g}")
                nc.scalar.activation(out=th[:, 0:nsub * N], in_=G.xmbuf[:, 0:nsub * N],
                                     func=Act.Tanh)
                for j in range(nsub):
                    ex = lp.tile([GB, N], F32, tag=f"ex{G.g}")
                    nc.scalar.activation(out=ex[:], in_=th[:, j * N:(j + 1) * N],
                                         func=Act.Exp, scale=10.0,
                                         accum_out=G.sscparts[:, t0 + j:t0 + j + 1])

            # ---------------- decode ----------------
            for G in grps:
                new_xmbuf(G)
                # x0 straight into xmbuf slot 0
                nc.sync.dma_start(out=xm_ap(G, 0), in_=bases_dram[G.g * GB:(G.g + 1) * GB, 1, :])
            for G in grps:
                emit_select(G, 0)
                emit_postselect(G, 0, n_steps, mask_bf=False)
                G.aidx0 = G.aidx

            for t in range(1, n_steps):
                for G in grps:
                    if t % K_BATCH == 0:
                        emit_act_batch(G, t // K_BATCH - 1, n_steps)
                        new_xmbuf(G)
                    g_t = G.gt_next
                    xm = xm_ap(G, t)
                    if t == 1:
                        # bf = base + G(a0);  bf[a0] += -1e9 came from eqf of step0?
                        # step0's eqf was computed against bf before it existed -> do here
                        nc.vector.tensor_tensor(out=G.bf[:], in0=G.base[:], in1=g_t[:],
                                                op=Alu.add)
                        af0 = lp.tile([GB, 1], F32, tag=f"af{G.g}")
                        nc.gpsimd.tensor_copy(out=af0[:], in_=G.aidx0[:, 0:1])
                        eqf0 = lp.tile([GB, N], F32, tag=f"eqf{G.g}")
                        nc.vector.tensor_scalar(out=eqf0[:], in0=iota_row[:],
                                                scalar1=af0[:, 0:1], scalar2=NEG_BIG,
                                                op0=Alu.is_equal, op1=Alu.mult)
                        nc.vector.tensor_tensor(out=G.bf[:], in0=G.bf[:], in1=eqf0[:],
                                                op=Alu.add)
                    nc.vector.tensor_tensor(out=xm, in0=G.bf[:], in1=g_t[:], op=Alu.add)
                    emit_select(G, t)
                    emit_postselect(G, t, n_steps)

            for G in grps:
                emit_act_batch(G, (n_steps - 1) // K_BATCH, n_steps)

            # ---------------- finalize logp ----------------
            for G in grps:
                ssc = lp.tile([GB, N], F32, tag=f"sscf{G.g}")
                nc.vector.tensor_tensor(out=ssc[:, 0:n_steps], in0=G.sscparts[:, 0:n_steps],
                                        in1=corr[:, 0:n_steps], op=Alu.subtract)
                lnp = lp.tile([GB, N], F32, tag=f"lnp{G.g}")
                nc.scalar.activation(out=lnp[:, 0:n_steps], in_=ssc[:, 0:n_steps], func=Act.Ln)
                thm = lp.tile([GB, N], F32, tag=f"thm{G.g}")
                nc.scalar.activation(out=thm[:, 0:n_steps], in_=G.m1parts[:, 0:n_steps],
                                     func=Act.Tanh)
                s1 = lp.tile([GB, 1], F32, tag=f"s1{G.g}")
                nc.vector.reduce_sum(out=s1[:], in_=thm[:, 0:n_steps], axis=AX_X)
                s2 = lp.tile([GB, 1], F32, tag=f"s2{G.g}")
                nc.vector.reduce_sum(out=s2[:], in_=lnp[:, 0:n_steps], axis=AX_X)
                lp_t = lp.tile([GB, 1], F32, tag=f"lpt{G.g}")
                nc.vector.tensor_scalar(out=lp_t[:], in0=s1[:], scalar1=10.0,
                                        scalar2=None, op0=Alu.mult)
                nc.vector.tensor_tensor(out=lp_t[:], in0=lp_t[:], in1=s2[:], op=Alu.subtract)
                nc.sync.dma_start(out=logp_out[G.g * GB:(G.g + 1) * GB, :], in_=lp_t[:])
                nc.sync.dma_start(out=pi_out[G.g * GB:(G.g + 1) * GB, :], in_=G.pi[:, 0:N])

    if compile:
        nc.compile()
    return nc


def _host_prep(hvec, hbar, qv_p, kv_p, vec_1, vec_f):
    """Host-side prep: scale fold + context projections + per-core input maps."""
    hvec = np.asarray(hvec, dtype=np.float32)
    hbar = np.asarray(hbar, dtype=np.float32)
    qf = np.asarray(qv_p, dtype=np.float32).reshape(D, D)
    kf = np.asarray(kv_p, dtype=np.float32).reshape(D, D)
    vec_1 = np.asarray(vec_1, dtype=np.float32)
    vec_f = np.asarray(vec_f, dtype=np.float32)

    qf_sc = (np.float32(0.25) * qf).astype(np.float32)
    qhbar_sc = (hbar @ qf_sc).astype(np.float32)
    ctx0 = ((hbar + vec_1[None, :]).astype(np.float32) + vec_f[None, :]).astype(np.float32)
    q0_sc = (ctx0 @ qf_sc).astype(np.float32)

    in_maps = []
    for c in range(N_CORES):
        sl = slice(c * BL, (c + 1) * BL)
        qh2 = np.empty((D, 2 * BL), np.float32)
        qh2[:, 0::2] = qhbar_sc[sl].T
        qh2[:, 1::2] = q0_sc[sl].T
        in_maps.append({
            "hvec": np.ascontiguousarray(hvec[sl]),
            "qf": qf_sc,
            "kf": np.ascontiguousarray(kf),
            "qh2": qh2,
        })
    return in_maps


def kernel(hvec, hbar, qv_p, kv_p, vec_1, vec_f):
    in_maps = _host_prep(hvec, hbar, qv_p, kv_p, vec_1, vec_f)
    nc = build_nc()
    res = run_bass_kernel_spmd(nc, in_maps, list(range(N_CORES)))
    pi = np.concatenate([np.asarray(r["pi"]) for r in res.results], axis=0)
    logp = np.concatenate([np.asarray(r["logp"]).reshape(-1) for r in res.results])
    return np.ascontiguousarray(pi.T.astype(np.int32)), logp.astype(np.float32)


# revision 22
# speedup vs baseline: 2.8156x; 1.1201x over previous
"""Trainium2 Bass kernel for pointer-network greedy decode (sparse_attention).

Problem: B=256 batches, N=384 nodes, D=128, H*Hd=128. Sequential greedy
decode with visited masking, tanh-clipped bilinear scores.

Key algebraic reformulation: scores sum over all heads/dims, so
    raw[b,n] = ctx[b]^T (qv_flat kv_flat^T) hvec[b,n]
with ctx = hbar + h_last + h_first.  Precompute per-batch pairwise matrix
    A[b] = (hvec[b] @ qf_sc) @ (hvec[b] @ kf)^T     (qf_sc = 0.25*qf)
Then each decode step is:  x = base[b] + A[b][a_last,:] + A[b][a_first,:]
(an indirect-DMA row gather + adds instead of matmuls).

Selection must reproduce XLA-CPU fp32 tanh tie semantics: tanh(x)==1.0 iff
x >= L_SAT (=7.9988117f). Selection rule (validated bit-exact vs reference
on the problem seed): first unvisited index with x >= T, where
    T = min(max_unvisited(x), L_SAT),  or -1e6 if max <= -L_SAT
Masking is additive (-1e9 into the persistent bf tile).

Performance structure:
- selection chain on DVE; mask update overlaps the next gather's latency.
- tanh/exp for logp are batched K_BATCH steps at a time (per-step ACT table
  loads otherwise dominate); ln/tanh-of-max batched once at the end.

Sharding: pure data-parallel over batch, 8 cores x 32 batches.
"""

import numpy as np

import concourse.bass as bass
import concourse.bacc as bacc
import concourse.mybir as mybir
from concourse.bass import IndirectOffsetOnAxis
from concourse.bass_utils import run_bass_kernel_spmd
from concourse.masks import make_identity
from concourse.tile import TileContext

F32 = mybir.dt.float32
U32 = mybir.dt.uint32
I32 = mybir.dt.int32
U8 = mybir.dt.uint8

B_FULL = 256
N_CORES = 8
BL = B_FULL // N_CORES  # 32 batches per core
N = 384
D = 128

K_BATCH = 8             # steps per ACT tanh batch

L_SAT = 7.9988117  # np.float32: smallest x with XLA-cpu tanh(x) == 1.0
NEG_BIG = -1.0e9
NEG_MED = -1.0e6

AX_X = mybir.AxisListType.X
Alu = mybir.AluOpType
Act = mybir.ActivationFunctionType


def build_nc(n_steps: int = N, compile: bool = True) -> bass.Bass:
    assert n_steps % K_BATCH == 0
    nc = bacc.Bacc()

    hvec_in = nc.dram_tensor("hvec", [BL, N, D], F32, kind="ExternalInput")
    qf_in = nc.dram_tensor("qf", [D, D], F32, kind="ExternalInput")   # pre-scaled by 0.25
    kf_in = nc.dram_tensor("kf", [D, D], F32, kind="ExternalInput")
    qh2_in = nc.dram_tensor("qh2", [D, 2 * BL], F32, kind="ExternalInput")

    pi_out = nc.dram_tensor("pi", [BL, N], I32, kind="ExternalOutput")
    logp_out = nc.dram_tensor("logp", [BL, 1], F32, kind="ExternalOutput")

    a_dram = nc.dram_tensor("a_mat", [BL * N, N], F32)
    bases_dram = nc.dram_tensor("bases", [BL, 2, N], F32)

    with TileContext(nc) as tc:
        with (
            tc.tile_pool(name="const", bufs=1) as cpool,
            tc.tile_pool(name="p1", bufs=2) as p1,
            tc.tile_pool(name="psum", bufs=2, space="PSUM") as psum,
            tc.tile_pool(name="loop", bufs=3) as lp,
            tc.tile_pool(name="xmb", bufs=2) as xmb,
            tc.tile_pool(name="pers", bufs=1) as pers,
        ):
            # ---------------- constants ----------------
            ident = cpool.tile([128, 128], F32, tag="ident")
            make_identity(nc, ident[:])
            qf_sb = cpool.tile([D, D], F32, tag="qf")
            nc.sync.dma_start(out=qf_sb[:], in_=qf_in[:])
            kf_sb = cpool.tile([D, D], F32, tag="kf")
            nc.sync.dma_start(out=kf_sb[:], in_=kf_in[:])
            qh2_sb = cpool.tile([D, 2 * BL], F32, tag="qh2")
            nc.sync.dma_start(out=qh2_sb[:], in_=qh2_in[:])

            # ---------------- phase 1: A = Qsc @ K^T per batch ----------------
            for b in range(BL):
                hv = p1.tile([128, 3, 128], F32, tag="hv")
                nc.sync.dma_start(out=hv[:], in_=hvec_in[b].rearrange("(c p) d -> p c d", p=128))
                hvT = p1.tile([128, N], F32, tag="hvT")
                for c in range(3):
                    tps = psum.tile([128, 128], F32, tag="tps")
                    nc.tensor.transpose(out=tps[:], in_=hv[:, c, :], identity=ident[:])
                    nc.vector.tensor_copy(out=hvT[:, c * 128:(c + 1) * 128], in_=tps[:])
                qt_ps = psum.tile([128, N], F32, tag="qkps")
                nc.tensor.matmul(out=qt_ps[:], lhsT=qf_sb[:], rhs=hvT[:], start=True, stop=True)
                qt = p1.tile([128, N], F32, tag="qt")
                nc.scalar.copy(out=qt[:], in_=qt_ps[:])
                kt_ps = psum.tile([128, N], F32, tag="qkps")
                nc.tensor.matmul(out=kt_ps[:], lhsT=kf_sb[:], rhs=hvT[:], start=True, stop=True)
                kt = p1.tile([128, N], F32, tag="kt")
                nc.scalar.copy(out=kt[:], in_=kt_ps[:])
                for c in range(3):
                    a_ps = psum.tile([128, N], F32, tag="aps")
                    nc.tensor.matmul(out=a_ps[:], lhsT=qt[:, c * 128:(c + 1) * 128],
                                     rhs=kt[:], start=True, stop=True)
                    a_sb = p1.tile([128, N], F32, tag="asb")
                    nc.vector.tensor_copy(out=a_sb[:], in_=a_ps[:])
                    nc.sync.dma_start(out=a_dram[b * N + c * 128: b * N + (c + 1) * 128, :],
                                      in_=a_sb[:])
                b_ps = psum.tile([2, N], F32, tag="bps")
                nc.tensor.matmul(out=b_ps[:], lhsT=qh2_sb[:, 2 * b:2 * b + 2],
                                 rhs=kt[:], start=True, stop=True)
                b_sb = p1.tile([2, N], F32, tag="bsb")
                nc.vector.tensor_copy(out=b_sb[:], in_=b_ps[:])
                nc.sync.dma_start(out=bases_dram[b], in_=b_sb[:])

            # ---------------- decode constants ----------------
            iota_u32 = pers.tile([BL, N], U32, tag="iotau")
            nc.gpsimd.iota(iota_u32[:], pattern=[[1, N]], base=0, channel_multiplier=0)
            iota_row = pers.tile([BL, N], F32, tag="iota")
            nc.vector.tensor_copy(out=iota_row[:], in_=iota_u32[:])
            rowbase = pers.tile([BL, 1], U32, tag="rowb")
            nc.gpsimd.iota(rowbase[:], pattern=[[0, 1]], base=0, channel_multiplier=N)
            ones8 = pers.tile([BL, 8], F32, tag="ones8")
            nc.vector.memset(ones8[:], 1.0)
            neg1e6 = pers.tile([BL, 1], F32, tag="neg1e6")
            nc.vector.memset(neg1e6[:], NEG_MED)
            negone = pers.tile([BL, 1], F32, tag="negone")
            nc.vector.memset(negone[:], -1.0)
            e10 = pers.tile([BL, 1], F32, tag="e10")          # device's exp(-10)
            nc.scalar.activation(out=e10[:], in_=negone[:], func=Act.Exp, scale=10.0)
            corr = pers.tile([BL, N], F32, tag="corr")        # corr[:, t] = t * e10
            nc.vector.tensor_scalar(out=corr[:], in0=iota_row[:], scalar1=e10[:, 0:1],
                                    scalar2=None, op0=Alu.mult)

            base_sb = pers.tile([BL, N], F32, tag="base")
            nc.sync.dma_start(out=base_sb[:], in_=bases_dram[:, 0, :])
            bf = pers.tile([BL, N], F32, tag="bf")
            pi_sb = pers.tile([BL, N], I32, tag="pi")
            nc.vector.memset(pi_sb[:], 0)
            m1parts = pers.tile([BL, N], F32, tag="m1p")
            sscparts = pers.tile([BL, N], F32, tag="ssc")

            state = {"xmbuf": None, "aidx": None, "gt": None}

            def new_xmbuf():
                state["xmbuf"] = xmb.tile([BL, K_BATCH * N], F32, tag="xmbuf", name="xmbuf")

            def xm_ap(t):
                o = (t % K_BATCH) * N
                return state["xmbuf"][:, o:o + N]

            def emit_select(t, n_steps):
                """DVE selection chain for step t (xm already in xmbuf) + gather issue."""
                xm = xm_ap(t)
                m1 = m1parts[:, t:t + 1]
                nc.vector.tensor_reduce(out=m1, in_=xm, axis=AX_X, op=Alu.max)
                thr = lp.tile([BL, 1], F32, tag="thr")
                nc.vector.tensor_scalar(out=thr[:], in0=m1, scalar1=L_SAT,
                                        scalar2=None, op0=Alu.min)
                negsat = lp.tile([BL, 1], U8, tag="negsat")
                nc.vector.tensor_scalar(out=negsat[:], in0=m1, scalar1=-L_SAT,
                                        scalar2=None, op0=Alu.is_le)
                nc.vector.copy_predicated(out=thr[:], mask=negsat[:], data=neg1e6[:])
                y = lp.tile([BL, N], F32, tag="y")
                nc.vector.tensor_scalar(out=y[:], in0=xm, scalar1=thr[:],
                                        scalar2=None, op0=Alu.is_ge)
                aidx = lp.tile([BL, 8], U32, tag="aidx")
                nc.vector.max_index(out=aidx[:], in_max=ones8[:], in_values=y[:])
                state["aidx"] = aidx
                # idx + gather issue FIRST (critical path), bookkeeping after
                if t < n_steps - 1:
                    idx = lp.tile([BL, 1], U32, tag="idx")
                    nc.vector.tensor_tensor(out=idx[:], in0=aidx[:, 0:1],
                                            in1=rowbase[:], op=Alu.add)
                    g_t = lp.tile([BL, N], F32, tag="g")
                    nc.gpsimd.indirect_dma_start(
                        out=g_t[:], out_offset=None, in_=a_dram[:],
                        in_offset=IndirectOffsetOnAxis(ap=idx[:, :1], axis=0))
                    state["gt"] = g_t
                nc.gpsimd.tensor_copy(out=pi_sb[:, t:t + 1], in_=aidx[:, 0:1])

            def emit_mask(aidx):
                """bf[b, a_b] += -1e9 (overlaps gather latency)."""
                af = lp.tile([BL, 1], F32, tag="af")
                nc.gpsimd.tensor_copy(out=af[:], in_=aidx[:, 0:1])
                eqf = lp.tile([BL, N], F32, tag="eqf")
                nc.vector.tensor_scalar(out=eqf[:], in0=iota_row[:], scalar1=af[:, 0:1],
                                        scalar2=NEG_BIG, op0=Alu.is_equal, op1=Alu.mult)
                nc.vector.tensor_tensor(out=bf[:], in0=bf[:], in1=eqf[:], op=Alu.add)

            def emit_act_batch(k, n_steps):
                t0 = k * K_BATCH
                nsub = min(K_BATCH, n_steps - t0)
                th = xmb.tile([BL, K_BATCH * N], F32, tag="th")
                nc.scalar.activation(out=th[:, 0:nsub * N], in_=state["xmbuf"][:, 0:nsub * N],
                                     func=Act.Tanh)
                for j in range(nsub):
                    ex = lp.tile([BL, N], F32, tag="ex")
                    nc.scalar.activation(out=ex[:], in_=th[:, j * N:(j + 1) * N],
                                         func=Act.Exp, scale=10.0,
                                         accum_out=sscparts[:, t0 + j:t0 + j + 1])

            # ---------------- decode ----------------
            new_xmbuf()
            nc.sync.dma_start(out=xm_ap(0), in_=bases_dram[:, 1, :])   # x0
            emit_select(0, n_steps)
            aidx0 = state["aidx"]

            for t in range(1, n_steps):
                if t % K_BATCH == 0:
                    emit_act_batch(t // K_BATCH - 1, n_steps)
                    new_xmbuf()
                g_t = state["gt"]
                xm = xm_ap(t)
                if t == 1:
                    nc.vector.tensor_tensor(out=bf[:], in0=base_sb[:], in1=g_t[:],
                                            op=Alu.add)
                    emit_mask(aidx0)
                nc.vector.tensor_tensor(out=xm, in0=bf[:], in1=g_t[:], op=Alu.add)
                emit_select(t, n_steps)
                if t < n_steps - 1:
                    emit_mask(state["aidx"])

            emit_act_batch((n_steps - 1) // K_BATCH, n_steps)

            # ---------------- finalize logp ----------------
            ssc = lp.tile([BL, N], F32, tag="sscf")
            nc.vector.tensor_tensor(out=ssc[:, 0:n_steps], in0=sscparts[:, 0:n_steps],
                                    in1=corr[:, 0:n_steps], op=Alu.subtract)
            lnp = lp.tile([BL, N], F32, tag="lnp")
            nc.scalar.activation(out=lnp[:, 0:n_steps], in_=ssc[:, 0:n_steps], func=Act.Ln)
            thm = lp.tile([BL, N], F32, tag="thm")
            nc.scalar.activation(out=thm[:, 0:n_steps], in_=m1parts[:, 0:n_steps],
                                 func=Act.Tanh)
            s1 = lp.tile([BL, 1], F32, tag="s1")
            nc.vector.reduce_sum(out=s1[:], in_=thm[:, 0:n_steps], axis=AX_X)
            s2 = lp.tile([BL, 1], F32, tag="s2")
            nc.vector.reduce_sum(out=s2[:], in_=lnp[:, 0:n_steps], axis=AX_X)
            lp_t = lp.tile([BL, 1], F32, tag="lpt")
            nc.vector.tensor_scalar(out=lp_t[:], in0=s1[:], scalar1=10.0,
                                    scalar2=None, op0=Alu.mult)
            nc.vector.tensor_tensor(out=lp_t[:], in0=lp_t[:], in1=s2[:], op=Alu.subtract)
            nc.sync.dma_start(out=logp_out[:], in_=lp_t[:])
            nc.sync.dma_start(out=pi_out[:], in_=pi_sb[:, 0:N])

    if compile:
        nc.compile()
    return nc


def _host_prep(hvec, hbar, qv_p, kv_p, vec_1, vec_f):
    """Host-side prep: scale fold + context projections + per-core input maps."""
    hvec = np.asarray(hvec, dtype=np.float32)
    hbar = np.asarray(hbar, dtype=np.float32)
    qf = np.asarray(qv_p, dtype=np.float32).reshape(D, D)
    kf = np.asarray(kv_p, dtype=np.float32).reshape(D, D)
    vec_1 = np.asarray(vec_1, dtype=np.float32)
    vec_f = np.asarray(vec_f, dtype=np.float32)

    qf_sc = (np.float32(0.25) * qf).astype(np.float32)
    qhbar_sc = (hbar @ qf_sc).astype(np.float32)
    ctx0 = ((hbar + vec_1[None, :]).astype(np.float32) + vec_f[None, :]).astype(np.float32)
    q0_sc = (ctx0 @ qf_sc).astype(np.float32)

    in_maps = []
    for c in range(N_CORES):
        sl = slice(c * BL, (c + 1) * BL)
        qh2 = np.empty((D, 2 * BL), np.float32)
        qh2[:, 0::2] = qhbar_sc[sl].T
        qh2[:, 1::2] = q0_sc[sl].T
        in_maps.append({
            "hvec": np.ascontiguousarray(hvec[sl]),
            "qf": qf_sc,
            "kf": np.ascontiguousarray(kf),
            "qh2": qh2,
        })
    return in_maps


def kernel(hvec, hbar, qv_p, kv_p, vec_1, vec_f):
    in_maps = _host_prep(hvec, hbar, qv_p, kv_p, vec_1, vec_f)
    nc = build_nc()
    res = run_bass_kernel_spmd(nc, in_maps, list(range(N_CORES)))
    pi = np.concatenate([np.asarray(r["pi"]) for r in res.results], axis=0)
    logp = np.concatenate([np.asarray(r["logp"]).reshape(-1) for r in res.results])
    return np.ascontiguousarray(pi.T.astype(np.int32)), logp.astype(np.float32)
